# revision 4
# baseline (speedup 1.0000x reference)
"""Trainium2 kernel for nn_ConvNN_2D_Spatial_K_N_Location.

Strategy (8 NeuronCores):
  - The two KNN-conv layers (irregular top-9 selection/gather, ~6% of FLOPs)
    run on host in fp32 with reference-exact tie-breaking.
  - The dominant FC stack runs on the 8 cores with the fc1 contraction dim
    (32768) sharded 8 ways in bf16: core i gets h2.T[F_i] and fw1.T[F_i]
    (8 MB each), computes fp32 partials for all 1024 batch rows, an
    on-device ReduceScatter leaves core i with final fc1 batch rows
    [128i:128(i+1)], then fused bias+relu and fc2 produce its 128x10 slice.
    H2D traffic is 128 MB bf16 total vs 1.15 GB for a replicated-fw1 plan
    (the host<->device link is the bottleneck at ~25-70 MB/s).
"""
import numpy as np
import ml_dtypes

import concourse.bass as bass
import concourse.tile as tile
from concourse import bacc, mybir
from concourse.bass_utils import run_bass_kernel_spmd

K, N, SCALE = 9, 8, 2
BF16 = np.dtype(ml_dtypes.bfloat16)
NCORES = 8
B = 1024
F = 32768
FSH = F // NCORES      # 4096
U = 1024
O2 = 10

_CACHE = {}


# ---------------------------------------------------------------- host conv
def _unshuffle(x, s):
    B_, C, H, W = x.shape
    return x.reshape(B_, C, H//s, s, W//s, s).transpose(0, 1, 3, 5, 2, 4).reshape(B_, C*s*s, H//s, W//s)


def _shuffle(x, s):
    B_, C, H, W = x.shape
    return x.reshape(B_, C//(s*s), s, s, H, W).transpose(0, 1, 4, 2, 5, 3).reshape(B_, C//(s*s), H*s, W*s)


def _conv_nn(x, w, b):
    x = _unshuffle(x, SCALE)
    B_, C, H, W = x.shape
    gy, gx = np.meshgrid(np.linspace(0., 1., H, dtype=np.float32),
                         np.linspace(0., 1., W, dtype=np.float32), indexing='ij')
    loc = np.broadcast_to(np.stack([gy, gx])[None], (B_, 2, H, W)).astype(np.float32)
    x = np.concatenate([x, loc], 1)
    Cf = C + 2
    xf = x.reshape(B_, Cf, H*W)
    ih = np.linspace(0, H-1, N).astype(np.int32)
    iw = np.linspace(0, W-1, N).astype(np.int32)
    samp = x[:, :, ih][:, :, :, iw].reshape(B_, Cf, N*N)
    # ranking key: d2 minus the per-token norm (constant in n, preserves order)
    s2 = np.einsum('bcn,bcn->bn', samp, samp)
    score = s2[:, None, :] - 2.0 * np.matmul(xf.transpose(0, 2, 1), samp)
    # top-K nearest, ties broken toward lower candidate index (== jax top_k)
    part = np.argpartition(score, K, axis=2)[:, :, :K]
    pv = np.take_along_axis(score, part, axis=2)
    o9 = np.lexsort((part, pv), axis=2)
    idx = np.take_along_axis(part, o9, axis=2)
    sampT = np.ascontiguousarray(samp.transpose(0, 2, 1))
    ng = sampT[np.arange(B_)[:, None, None], idx, :]        # (B, T, K, Cf)
    w_kc = np.ascontiguousarray(w.transpose(0, 2, 1)).reshape(w.shape[0], K * Cf)
    out = ng.reshape(B_ * H * W, K * Cf) @ w_kc.T
    out += b
    out = out.reshape(B_, H * W, w.shape[0]).transpose(0, 2, 1)
    return _shuffle(out.reshape(B_, w.shape[0], H, W), SCALE)


# ---------------------------------------------------------------- device fc
def _build_fc_kernel():
    if 'nc' in _CACHE:
        return _CACHE['nc']
    nc = bacc.Bacc("TRN2", target_bir_lowering=False, debug=False,
                   enable_asserts=False, num_devices=NCORES)
    f32 = mybir.dt.float32
    bf16 = mybir.dt.bfloat16
    h2ti = nc.dram_tensor("h2ti", (FSH, B), bf16, kind="ExternalInput").ap()
    fw1s = nc.dram_tensor("fw1s", (FSH, U), bf16, kind="ExternalInput").ap()
    fb1t = nc.dram_tensor("fb1t", (128, 8), f32, kind="ExternalInput").ap()
    fw2t = nc.dram_tensor("fw2t", (U, O2), bf16, kind="ExternalInput").ap()
    fb2r = nc.dram_tensor("fb2r", (1, O2), bf16, kind="ExternalInput").ap()
    onesr = nc.dram_tensor("onesr", (1, 128), bf16, kind="ExternalInput").ap()
    ident = nc.dram_tensor("ident", (128, 128), f32, kind="ExternalInput").ap()
    outt = nc.dram_tensor("outt", (128, O2), f32, kind="ExternalOutput").ap()

    NCH = FSH // 128       # 32 feature chunks per core

    with tile.TileContext(nc) as tc:
        with tc.tile_pool(name="wres", bufs=1) as wres, \
             tc.tile_pool(name="small", bufs=1) as spool, \
             tc.tile_pool(name="stage", bufs=2) as stpool, \
             tc.tile_pool(name="acts", bufs=1) as apool, \
             tc.tile_pool(name="ps", bufs=2, space="PSUM") as pspool, \
             tc.tile_pool(name="pst", bufs=2, space="PSUM") as ptpool, \
             tc.tile_pool(name="dram", bufs=1, space="DRAM") as dram:

            # resident weights + activations: 64KB + 64KB per partition
            wtile = wres.tile([128, NCH * U], bf16)
            htile = wres.tile([128, NCH * B], bf16)
            for c in range(NCH):
                nc.sync.dma_start(wtile[:, bass.ts(c, U)], fw1s[bass.ts(c, 128), :])
                nc.sync.dma_start(htile[:, bass.ts(c, B)], h2ti[bass.ts(c, 128), :])

            ones_t = spool.tile([1, 128], bf16)
            nc.sync.dma_start(ones_t[:], onesr[:, :])
            fb1_t = spool.tile([128, 8], f32)
            nc.sync.dma_start(fb1_t[:], fb1t[:, :])
            fb2_t = spool.tile([1, O2], bf16)
            nc.sync.dma_start(fb2_t[:], fb2r[:, :])
            id_t = spool.tile([128, 128], f32)
            nc.sync.dma_start(id_t[:], ident[:, :])
            fw2_t = spool.tile([128, 8 * O2], bf16)
            for c in range(8):
                nc.sync.dma_start(fw2_t[:, bass.ts(c, O2)], fw2t[bass.ts(c, 128), :])

            bounce_in = dram.tile([B, U], f32)
            bounce_out = dram.tile([128, U], f32)

            # fc1 partials over all 8 batch blocks
            for j in range(NCORES):
                psum = pspool.tile([128, U], f32)
                for c in range(NCH):
                    lhsT = htile[:, c * B + j * 128: c * B + (j + 1) * 128]
                    for half in range(2):
                        nc.tensor.matmul(psum[:, bass.ts(half, 512)],
                                         lhsT=lhsT,
                                         rhs=wtile[:, c * U + half * 512: c * U + (half + 1) * 512],
                                         start=(c == 0), stop=(c == NCH - 1))
                stg = stpool.tile([128, U], f32)
                nc.scalar.copy(stg[:], psum[:])
                nc.sync.dma_start(bounce_in[j * 128:(j + 1) * 128, :], stg[:])

            nc.gpsimd.collective_compute(
                "ReduceScatter", mybir.AluOpType.add,
                replica_groups=[list(range(NCORES))],
                ins=[bounce_in.opt()], outs=[bounce_out.opt()],
            )

            h1raw = apool.tile([128, U], f32)
            nc.sync.dma_start(h1raw[:], bounce_out[:])

            # transpose 128x128 blocks; relu(x + fb1) fused on the way out
            h1T = apool.tile([128, U], bf16)
            for c in range(8):
                pt = ptpool.tile([128, 128], f32)
                nc.tensor.transpose(pt[:], h1raw[:, bass.ts(c, 128)], id_t[:])
                nc.scalar.activation(h1T[:, bass.ts(c, 128)], pt[:],
                                     mybir.ActivationFunctionType.Relu,
                                     bias=fb1_t[:, c:c + 1])

            psum2 = ptpool.tile([128, O2], f32)
            for c in range(8):
                nc.tensor.matmul(psum2[:], lhsT=h1T[:, bass.ts(c, 128)],
                                 rhs=fw2_t[:, bass.ts(c, O2)],
                                 start=(c == 0), stop=False)
            nc.tensor.matmul(psum2[:], lhsT=ones_t[:], rhs=fb2_t[:],
                             start=False, stop=True)

            out_t = apool.tile([128, O2], f32)
            nc.scalar.copy(out_t[:], psum2[:])
            nc.sync.dma_start(outt[:, :], out_t[:])

    nc.compile()
    _CACHE['nc'] = nc
    return nc


def kernel(x, w1, b1, w2, b2, fw1, fb1, fw2, fb2):
    x = np.asarray(x, np.float32)
    # host: the two KNN-conv layers (exact fp32 ranking, reference tie-break)
    h1 = np.maximum(_conv_nn(x, np.asarray(w1, np.float32), np.asarray(b1, np.float32)), 0)
    h2 = np.maximum(_conv_nn(h1, np.asarray(w2, np.float32), np.asarray(b2, np.float32)), 0)
    h2 = h2.reshape(B, -1)                              # (1024, 32768)

    nc = _build_fc_kernel()
    h2t = h2.astype(BF16).T                             # (32768, 1024) bf16 view
    fw1t = np.asarray(fw1, np.float32).astype(BF16).T   # (32768, 1024) bf16 view
    fb1t = np.ascontiguousarray(np.asarray(fb1, np.float32).reshape(8, 128).T)
    fw2t = np.asarray(fw2, np.float32).T.astype(BF16)
    fb2r = np.asarray(fb2, np.float32).reshape(1, O2).astype(BF16)
    onesr = np.ones((1, 128), BF16)
    ident = np.eye(128, dtype=np.float32)
    in_maps = []
    for i in range(NCORES):
        sl = slice(i * FSH, (i + 1) * FSH)
        in_maps.append(dict(h2ti=np.ascontiguousarray(h2t[sl]),
                            fw1s=np.ascontiguousarray(fw1t[sl]),
                            fb1t=fb1t, fw2t=fw2t, fb2r=fb2r,
                            onesr=onesr, ident=ident))
    res = run_bass_kernel_spmd(nc, in_maps, core_ids=list(range(NCORES)))
    out = np.empty((B, O2), np.float32)
    for i in range(NCORES):
        out[i * 128:(i + 1) * 128] = res.results[i]["outt"]
    return out


# revision 7
# speedup vs baseline: 3.1105x; 3.1105x over previous
"""Trainium2 kernel for nn_ConvNN_2D_Spatial_K_N_Location.

Strategy (8 NeuronCores):
  - The two KNN-conv layers (irregular top-9 selection/gather, ~6% of FLOPs)
    run on host in fp32 with reference-exact tie-breaking.
  - The dominant FC stack runs on the 8 cores with the fc1 contraction dim
    (32768) sharded 8 ways in bf16: core i gets h2.T[F_i] and fw1.T[F_i]
    (8 MB each), computes fp32 partials for all 1024 batch rows, an
    on-device ReduceScatter leaves core i with final fc1 batch rows
    [128i:128(i+1)], then fused bias+relu and fc2 produce its 128x10 slice.
    H2D traffic is 128 MB bf16 total vs 1.15 GB for a replicated-fw1 plan
    (the host<->device link is the bottleneck at ~25-70 MB/s).
"""
import numpy as np
import ml_dtypes

import concourse.bass as bass
import concourse.tile as tile
from concourse import bacc, mybir
from concourse.bass_utils import run_bass_kernel_spmd

K, N, SCALE = 9, 8, 2
BF16 = np.dtype(ml_dtypes.bfloat16)
NCORES = 8
B = 1024
F = 32768
FSH = F // NCORES      # 4096
U = 1024
O2 = 10

_CACHE = {}


# ---------------------------------------------------------------- host conv
def _unshuffle(x, s):
    B_, C, H, W = x.shape
    return x.reshape(B_, C, H//s, s, W//s, s).transpose(0, 1, 3, 5, 2, 4).reshape(B_, C*s*s, H//s, W//s)


def _shuffle(x, s):
    B_, C, H, W = x.shape
    return x.reshape(B_, C//(s*s), s, s, H, W).transpose(0, 1, 4, 2, 5, 3).reshape(B_, C//(s*s), H*s, W*s)


def _conv_nn(x, w, b):
    x = _unshuffle(x, SCALE)
    B_, C, H, W = x.shape
    gy, gx = np.meshgrid(np.linspace(0., 1., H, dtype=np.float32),
                         np.linspace(0., 1., W, dtype=np.float32), indexing='ij')
    loc = np.broadcast_to(np.stack([gy, gx])[None], (B_, 2, H, W)).astype(np.float32)
    x = np.concatenate([x, loc], 1)
    Cf = C + 2
    xf = x.reshape(B_, Cf, H*W)
    ih = np.linspace(0, H-1, N).astype(np.int32)
    iw = np.linspace(0, W-1, N).astype(np.int32)
    samp = x[:, :, ih][:, :, :, iw].reshape(B_, Cf, N*N)
    # ranking key: d2 minus the per-token norm (constant in n, preserves order)
    s2 = np.einsum('bcn,bcn->bn', samp, samp)
    score = s2[:, None, :] - 2.0 * np.matmul(xf.transpose(0, 2, 1), samp)
    # top-K nearest, ties broken toward lower candidate index (== jax top_k)
    part = np.argpartition(score, K, axis=2)[:, :, :K]
    pv = np.take_along_axis(score, part, axis=2)
    o9 = np.lexsort((part, pv), axis=2)
    idx = np.take_along_axis(part, o9, axis=2)
    sampT = np.ascontiguousarray(samp.transpose(0, 2, 1))
    ng = sampT[np.arange(B_)[:, None, None], idx, :]        # (B, T, K, Cf)
    w_kc = np.ascontiguousarray(w.transpose(0, 2, 1)).reshape(w.shape[0], K * Cf)
    out = ng.reshape(B_ * H * W, K * Cf) @ w_kc.T
    out += b
    out = out.reshape(B_, H * W, w.shape[0]).transpose(0, 2, 1)
    return _shuffle(out.reshape(B_, w.shape[0], H, W), SCALE)


# ---------------------------------------------------------------- device fc
def _build_fc_kernel():
    if 'nc' in _CACHE:
        return _CACHE['nc']
    nc = bacc.Bacc("TRN2", target_bir_lowering=False, debug=False,
                   enable_asserts=False, num_devices=NCORES)
    f32 = mybir.dt.float32
    bf16 = mybir.dt.bfloat16
    h2ti = nc.dram_tensor("h2ti", (FSH, B), bf16, kind="ExternalInput").ap()
    fw1s = nc.dram_tensor("fw1s", (FSH, U), bf16, kind="ExternalInput").ap()
    fb1t = nc.dram_tensor("fb1t", (128, 8), f32, kind="ExternalInput").ap()
    fw2t = nc.dram_tensor("fw2t", (U, O2), bf16, kind="ExternalInput").ap()
    fb2r = nc.dram_tensor("fb2r", (1, O2), bf16, kind="ExternalInput").ap()
    onesr = nc.dram_tensor("onesr", (1, 128), bf16, kind="ExternalInput").ap()
    ident = nc.dram_tensor("ident", (128, 128), f32, kind="ExternalInput").ap()
    outt = nc.dram_tensor("outt", (128, O2), f32, kind="ExternalOutput").ap()

    NCH = FSH // 128       # 32 feature chunks per core

    with tile.TileContext(nc) as tc:
        with tc.tile_pool(name="wres", bufs=1) as wres, \
             tc.tile_pool(name="small", bufs=1) as spool, \
             tc.tile_pool(name="stage", bufs=2) as stpool, \
             tc.tile_pool(name="acts", bufs=1) as apool, \
             tc.tile_pool(name="ps", bufs=2, space="PSUM") as pspool, \
             tc.tile_pool(name="pst", bufs=2, space="PSUM") as ptpool, \
             tc.tile_pool(name="dram", bufs=1, space="DRAM") as dram:

            # resident weights + activations: 64KB + 64KB per partition
            wtile = wres.tile([128, NCH * U], bf16)
            htile = wres.tile([128, NCH * B], bf16)
            for c in range(NCH):
                nc.sync.dma_start(wtile[:, bass.ts(c, U)], fw1s[bass.ts(c, 128), :])
                nc.sync.dma_start(htile[:, bass.ts(c, B)], h2ti[bass.ts(c, 128), :])

            ones_t = spool.tile([1, 128], bf16)
            nc.sync.dma_start(ones_t[:], onesr[:, :])
            fb1_t = spool.tile([128, 8], f32)
            nc.sync.dma_start(fb1_t[:], fb1t[:, :])
            fb2_t = spool.tile([1, O2], bf16)
            nc.sync.dma_start(fb2_t[:], fb2r[:, :])
            id_t = spool.tile([128, 128], f32)
            nc.sync.dma_start(id_t[:], ident[:, :])
            fw2_t = spool.tile([128, 8 * O2], bf16)
            for c in range(8):
                nc.sync.dma_start(fw2_t[:, bass.ts(c, O2)], fw2t[bass.ts(c, 128), :])

            bounce_in = dram.tile([B, U], f32)
            bounce_out = dram.tile([128, U], f32)

            # fc1 partials over all 8 batch blocks
            for j in range(NCORES):
                psum = pspool.tile([128, U], f32)
                for c in range(NCH):
                    lhsT = htile[:, c * B + j * 128: c * B + (j + 1) * 128]
                    for half in range(2):
                        nc.tensor.matmul(psum[:, bass.ts(half, 512)],
                                         lhsT=lhsT,
                                         rhs=wtile[:, c * U + half * 512: c * U + (half + 1) * 512],
                                         start=(c == 0), stop=(c == NCH - 1))
                stg = stpool.tile([128, U], f32)
                nc.scalar.copy(stg[:], psum[:])
                nc.sync.dma_start(bounce_in[j * 128:(j + 1) * 128, :], stg[:])

            nc.gpsimd.collective_compute(
                "ReduceScatter", mybir.AluOpType.add,
                replica_groups=[list(range(NCORES))],
                ins=[bounce_in.opt()], outs=[bounce_out.opt()],
            )

            h1raw = apool.tile([128, U], f32)
            nc.sync.dma_start(h1raw[:], bounce_out[:])

            # transpose 128x128 blocks; relu(x + fb1) fused on the way out
            h1T = apool.tile([128, U], bf16)
            for c in range(8):
                pt = ptpool.tile([128, 128], f32)
                nc.tensor.transpose(pt[:], h1raw[:, bass.ts(c, 128)], id_t[:])
                nc.scalar.activation(h1T[:, bass.ts(c, 128)], pt[:],
                                     mybir.ActivationFunctionType.Relu,
                                     bias=fb1_t[:, c:c + 1])

            psum2 = ptpool.tile([128, O2], f32)
            for c in range(8):
                nc.tensor.matmul(psum2[:], lhsT=h1T[:, bass.ts(c, 128)],
                                 rhs=fw2_t[:, bass.ts(c, O2)],
                                 start=(c == 0), stop=False)
            nc.tensor.matmul(psum2[:], lhsT=ones_t[:], rhs=fb2_t[:],
                             start=False, stop=True)

            out_t = apool.tile([128, O2], f32)
            nc.scalar.copy(out_t[:], psum2[:])
            nc.sync.dma_start(outt[:, :], out_t[:])

    nc.compile()
    _CACHE['nc'] = nc
    return nc


def kernel(x, w1, b1, w2, b2, fw1, fb1, fw2, fb2):
    import time as _time
    import sys as _sys
    _t0 = _time.time()

    def _mark(label):
        print(f"[kernel] {label}: {_time.time() - _t0:.2f}s", file=_sys.stderr, flush=True)

    x = np.asarray(x, np.float32)
    # host: the two KNN-conv layers (exact fp32 ranking, reference tie-break)
    h1 = np.maximum(_conv_nn(x, np.asarray(w1, np.float32), np.asarray(b1, np.float32)), 0)
    _mark("conv1")
    h2 = np.maximum(_conv_nn(h1, np.asarray(w2, np.float32), np.asarray(b2, np.float32)), 0)
    _mark("conv2")
    h2 = h2.reshape(B, -1)                              # (1024, 32768)

    nc = _build_fc_kernel()
    _mark("bass build+compile")
    h2t = h2.astype(BF16).T                             # (32768, 1024) bf16 view
    fw1t = np.asarray(fw1, np.float32).astype(BF16).T   # (32768, 1024) bf16 view
    fb1t = np.ascontiguousarray(np.asarray(fb1, np.float32).reshape(8, 128).T)
    fw2t = np.asarray(fw2, np.float32).T.astype(BF16)
    fb2r = np.asarray(fb2, np.float32).reshape(1, O2).astype(BF16)
    onesr = np.ones((1, 128), BF16)
    ident = np.eye(128, dtype=np.float32)
    in_maps = []
    for i in range(NCORES):
        sl = slice(i * FSH, (i + 1) * FSH)
        in_maps.append(dict(h2ti=np.ascontiguousarray(h2t[sl]),
                            fw1s=np.ascontiguousarray(fw1t[sl]),
                            fb1t=fb1t, fw2t=fw2t, fb2r=fb2r,
                            onesr=onesr, ident=ident))
    _mark("prep in_maps")
    res = run_bass_kernel_spmd(nc, in_maps, core_ids=list(range(NCORES)))
    _mark("spmd run")
    out = np.empty((B, O2), np.float32)
    for i in range(NCORES):
        out[i * 128:(i + 1) * 128] = res.results[i]["outt"]
    return out
